# revision 34
# baseline (speedup 1.0000x reference)
"""Deformable Conv2d on 8 Trainium2 NeuronCores.

Sharding: core k -> (batch b = k//2, image row-half yh = k%2).
Each core handles 2048 output pixels (32 rows x 64 cols), all 9 taps,
full C=256 / F=256.

Per-core device pipeline (bf16 compute, f32 psum accumulation):
  1. cast x[b] -> bf16 through SBUF (DVE cast), write DRAM xbf (padded,
     zeroed tail rows), then build xquad[r] = [x[r], x[r+1], x[r+64],
     x[r+65]] (the 2x2 bilinear footprint, 2KB/row) with 4 strided
     HWDGE DMAs.
  2. coords/weights/index from offsets on DVE (exact floor with
     round-up correction; tap grid matches the reference's meshgrid
     quirk). One int16 index plane (y0*64+x0), shuffled into the
     SWDGE wrapped [16, n/16] layout via a small DRAM round trip.
  3. gpsimd.dma_gather from xquad: ONE 2KB descriptor per (pixel, tap)
     -> [128 samples, 4 corners x 256c]. 36 calls of 512 indices on a
     single SWDGE queue. Real-HW SWDGE cost is ~10ns/descriptor/queue
     (descriptor-count bound, measured), so the 2KB quad descriptors
     halve gather queue time vs the 2-descriptor scheme (~184us,
     overlapped with the rest of the pipeline). Multi-queue would cut
     this further (~31us measured) but is unsound: Tile rotates Pool
     DMAs over the 8 DMASW sem lanes in scheduled order, queue-blind,
     while the ucode hard-locks each lane to its first queue -- any
     multi-queue assignment is a per-build coin flip (W is loaded via
     HWDGE + DVE cast for the same reason).
  4. bilinear blend px-major on ACT+DVE (1 activation-copy with
     per-partition scale + 3 fused scalar_tensor_tensor MACs).
  5. deform^T via PE transpose (identity matmul into PSUM + copy out).
     The HWDGE xbar dma_start_transpose would be cheaper but posts its
     completion semaphore per xbar sub-tile on real HW (CoreSim models
     it as atomic), so stage-2 consumers race build-dependently --
     observed as intermittent j-granular corruption. PE is sound.
  6. stage 2 computes out^T: psum[f128, px512] += W[n,ch][c,f] as
     resident lhsT @ deformT[c, px] rhs (N=512 streams, 144 matmuls,
     largely hidden under the DMA/blend pipeline). Host transposes
     [F, PX] back during unshard.
Bias is added on host during unshard (zeros in this problem).
Measured on HW (slope method below): ~302us/iteration vs a 225us
TimelineSim model and a ~176us per-core HBM roofline (~63MB traffic
at 358GB/s).

`_build_bass(reps=R)` unrolls the whole per-exec pipeline R times
(shared DRAM scratch serializes reps); test.py uses an A/B pair of
such modules to measure true on-device per-iteration time as a slope,
cancelling the ~85ms axon dispatch round-trip that dominates any
single blocking execution.
"""

import numpy as np

B, IH, IW, C = 4, 64, 64, 256
KH, KW, F = 3, 3, 256
N = KH * KW
HALF = IH // 2           # 32 rows per core
PX = HALF * IW           # 2048 pixels per core
NJ = PX // 128           # 16 column-tiles of 128 pixels
NPLANE = N * NJ          # 144
NCORES = 8

_cache = {}


def _host_consts(yh):
    # base grid planes [128, N*NJ]: col (n, j), partition p, pixel = j*128+p
    # Tap grid offsets reproduce the reference's meshgrid-stack-reshape quirk.
    flat = np.array([0, 0, 0, 1, 1, 1, 2, 2, 2, 0, 1, 2, 0, 1, 2, 0, 1, 2])
    DY = flat[0::2]
    DX = flat[1::2]
    p = np.arange(128)
    j = np.arange(NJ)
    px = j[None, :] * 128 + p[:, None]          # [128, NJ] local pixel id
    Y = yh * HALF + px // IW                    # global row
    X = px % IW
    baseY = (Y[:, None, :] - 1 + DY[None, :, None]).astype(np.float32)
    baseX = (X[:, None, :] - 1 + DX[None, :, None]).astype(np.float32)
    return baseY.reshape(128, N * NJ), baseX.reshape(128, N * NJ)


def _audit_pool_lanes(nc):
    """Return (issue->scheduled-index map OK?, per-issue queue fix).

    Tile assigns Pool DMA instructions to the 8 DMASW sem lanes round-robin
    in FINAL SCHEDULED order (queue-blind), and the SWDGE ucode hard-locks
    each sem lane to the queue of its first use. So queue_num must equal
    (scheduled pool-DMA index) % 4 -- which is only knowable after
    scheduling. _build_bass records issue-ordered gather instructions in
    nc._pool_dma_insts; this reads the scheduled order back.
    """
    import concourse.mybir as mybir
    fn = nc.m.functions[0]
    insts = []
    for blk in fn.blocks:
        insts.extend(blk.instructions)
    name_to_issue = {b.ins.name: i for i, b in enumerate(nc._pool_dma_insts)}
    sched = [ins for ins in insts
             if getattr(ins, 'engine', None) == mybir.EngineType.Pool
             and type(ins).__name__ == 'InstDMAGatherAnt']
    qfix = {}
    ok = len(sched) == len(name_to_issue)
    lanes = {}
    for si, ins in enumerate(sched):
        issue = name_to_issue.get(ins.name)
        if issue is None:
            ok = False
            continue
        qfix[issue] = si % 4
        lane = si % 8
        q = getattr(ins, 'queue_num', 0)
        if lanes.setdefault(lane, q) != q:
            ok = False
    return ok, qfix


def _build_bass_fixed(reps=1):
    """Build with SWDGE queue assignments consistent with the scheduled
    sem-lane rotation (fixed-point iteration; falls back to single-queue
    if the schedule refuses to converge)."""
    qmap = None
    for _ in range(4):
        nc = _build_bass(reps=reps, qmap=qmap)
        ok, qfix = _audit_pool_lanes(nc)
        if ok:
            return nc
        qmap = qfix
    return _build_bass(reps=reps, qmap="single")


def _build_bass(reps=1, qmap=None):
    import os
    import concourse.bass as bass
    import concourse.mybir as mybir
    import concourse.tile as tile
    from concourse import bacc
    from concourse import library_config

    ABL = os.environ.get("BASS_ABLATE", "")  # timing-only stage ablations
    dt = mybir.dt
    Alu = mybir.AluOpType
    nc = bacc.Bacc(None, target_bir_lowering=False,
                   dynamic_dma_scratch_size=32768, num_swdge_queues=1)

    xin = nc.dram_tensor("x", [IH * IW, C], dt.float32, kind="ExternalInput")
    offs_in = nc.dram_tensor("offs", [PX, 2 * N], dt.float32, kind="ExternalInput")
    w_in = nc.dram_tensor("w", [N, C, F], dt.float32, kind="ExternalInput")
    baseY_in = nc.dram_tensor("baseY", [128, NPLANE], dt.float32, kind="ExternalInput")
    baseX_in = nc.dram_tensor("baseX", [128, NPLANE], dt.float32, kind="ExternalInput")
    ident_in = nc.dram_tensor("ident", [128, 128], dt.bfloat16, kind="ExternalInput")
    out_t = nc.dram_tensor("out", [F, PX], dt.float32, kind="ExternalOutput")

    XROWS = IH * IW          # 4096
    PAD = 65                 # xbf tail rows zeroed (r+65 reads for r=4095)
    NW = N * 128             # 1152 wrapped idx columns

    with tile.TileContext(nc) as tc:
        with tc.tile_pool(name="dram", bufs=1, space="DRAM") as dpool:
            xbf_dram = dpool.tile([XROWS + PAD, C], dt.bfloat16)
            xquad_dram = dpool.tile([XROWS, 4 * C], dt.bfloat16)
            idx_dram = dpool.tile([128 * NPLANE], dt.int16)

            with tc.tile_pool(name="main", bufs=1) as pool:
                nc.gpsimd.load_library(library_config.attnmlp)
                # Warm the Q7 library IRAM (~6us) before the real gathers.
                warm_idx = pool.tile([128, 8], dt.int16)
                warm_out = pool.tile([128, 1, 64], dt.float32)
                nc.vector.memset(warm_idx[:], 0)
                # Pool-engine DMA instructions are assigned to the 8 DMASW
                # sem lanes round-robin in program order, and each lane
                # hard-locks to the SWDGE queue of its first use. Keeping
                # queue_num == (pool-dma counter) % 4 makes lane<->queue
                # consistent (8 lanes, 4 queues). pooldma[0] is the counter.
                pooldma = [0]
                nc._pool_dma_insts = []

                def next_q():
                    # Single SWDGE queue: Tile rotates Pool DMAs over the 8
                    # DMASW sem lanes in *scheduled* order (queue-blind) and
                    # the ucode hard-locks each lane to its first queue.
                    # The schedule is not stable across builds, so any
                    # multi-queue assignment is a per-build coin flip
                    # (observed as intermittent j-granular corruption).
                    # One queue can never conflict. The gathers serialize to
                    # ~184us on the queue but overlap the rest of the
                    # pipeline.
                    pooldma[0] += 1
                    return 0

                def track(inst):
                    nc._pool_dma_insts.append(inst)
                    return inst

                track(nc.gpsimd.dma_gather(
                    out_ap=warm_out[:],
                    in_ap=bass.AP(xin, 0, [[64, 128], [1, 64]]),
                    idxs_ap=warm_idx[:],
                    num_idxs=128,
                    num_idxs_reg=128,
                    elem_size=64,
                    elem_step=64,
                    queue_num=next_q(),
                ))
                zpad = pool.tile([128, C], dt.bfloat16)
                nc.vector.memset(zpad[:], 0.0)
                ident = pool.tile([128, 128], dt.bfloat16)
                nc.sync.dma_start(ident[:], ident_in[:])

                # persistent per-rep tiles (reused across reps)
                wb = pool.tile([128, N, 2, F], dt.bfloat16)
                baseY = pool.tile([128, NPLANE], dt.float32)
                baseX = pool.tile([128, NPLANE], dt.float32)
                offs = pool.tile([128, NJ, 2 * N], dt.float32)
                cy = pool.tile([128, NPLANE], dt.float32)
                cx = pool.tile([128, NPLANE], dt.float32)
                fy = pool.tile([128, NPLANE], dt.float32)
                fx = pool.tile([128, NPLANE], dt.float32)
                y0 = pool.tile([128, NPLANE], dt.float32)
                x0 = pool.tile([128, NPLANE], dt.float32)
                uy = pool.tile([128, NPLANE], dt.float32)
                vx = pool.tile([128, NPLANE], dt.float32)
                w00 = pool.tile([128, NPLANE], dt.float32)
                w01 = pool.tile([128, NPLANE], dt.float32)
                w10 = pool.tile([128, NPLANE], dt.float32)
                w11 = pool.tile([128, NPLANE], dt.float32)
                idxc = pool.tile([128, NPLANE], dt.int16)
                idf = pool.tile([128, NPLANE], dt.float32)
                itmp = pool.tile([128, NPLANE], dt.int32)
                neg = pool.tile([128, NPLANE], dt.float32)
                idxw = pool.tile([128, NW], dt.int16)

                def offview(d):
                    return bass.AP(offs.tensor, offs[:].offset + d,
                                   [[offs[:].ap[0][0], 128], [2, N], [2 * N, NJ]])

                def floor_into(dst_i, dst_f, src):
                    nc.vector.tensor_copy(itmp[:], src)
                    nc.vector.tensor_copy(dst_i[:], itmp[:])
                    nc.vector.tensor_tensor(dst_f[:], src, dst_i[:], Alu.subtract)
                    nc.vector.tensor_scalar(neg[:], dst_f[:], 0.0, None, Alu.is_lt)
                    nc.vector.tensor_tensor(dst_i[:], dst_i[:], neg[:], Alu.subtract)
                    nc.vector.tensor_tensor(dst_f[:], dst_f[:], neg[:], Alu.add)

                with (
                    tc.tile_pool(name="xstage", bufs=1) as xpool,
                    tc.tile_pool(name="gpool", bufs=2) as gpool,
                    tc.tile_pool(name="dfpool", bufs=1) as dfpool,
                    tc.tile_pool(name="pspool", bufs=1, space="PSUM") as pspool,
                    tc.tile_pool(name="tppool", bufs=1, space="PSUM") as tppool,
                    tc.tile_pool(name="ost", bufs=2) as opool,
                ):
                    deformT = pool.tile([128, 2, N, NJ, 128], dt.bfloat16)
                    xquadview = bass.AP(xquad_dram.tensor, 0,
                                        [[4 * C, XROWS], [1, 4 * C]])

                    for rep in range(reps):
                        SKIP_G = "gather" in ABL
                        SKIP_B = "blend" in ABL
                        SKIP_T = "tpose" in ABL
                        SKIP_M = "mm" in ABL
                        SKIP_P = "prefix" in ABL
                        # ---- constant/weight loads (counted per rep) ----
                        # W: HWDGE f32 load + DVE cast (keeps Pool DMA lanes
                        # exclusively for gathers -- see next_q()).
                        for ck in range(3):
                            wf = xpool.tile([128, 6, F], dt.float32,
                                            tag=f"wf{ck % 2}")
                            nc.scalar.dma_start(
                                wf[:],
                                bass.AP(w_in, ck * 6 * 128 * F,
                                        [[F, 128], [128 * F, 6], [1, F]]),
                            )
                            nc.vector.tensor_copy(
                                bass.AP(wb.tensor,
                                        wb[:].offset + ck * 6 * F,
                                        [[wb[:].ap[0][0], 128], [F, 6], [1, F]]),
                                wf[:])
                        nc.scalar.dma_start(baseY[:], baseY_in[:])
                        nc.scalar.dma_start(baseX[:], baseX_in[:])
                        nc.scalar.dma_start(
                            offs[:],
                            bass.AP(offs_in, 0,
                                    [[2 * N, 128], [128 * 2 * N, NJ], [1, 2 * N]]),
                        )

                        # ---- x -> bf16 (DVE cast through SBUF) ----
                        for ck in ([] if SKIP_P else range(4)):
                            xf = xpool.tile([128, 8, C], dt.float32, tag=f"xf{ck % 2}")
                            xb = xpool.tile([128, 8, C], dt.bfloat16, tag=f"xb{ck % 2}")
                            eng = nc.sync if ck % 2 == 0 else nc.scalar
                            eng.dma_start(
                                xf[:],
                                bass.AP(xin, ck * 1024 * C,
                                        [[C, 128], [128 * C, 8], [1, C]]),
                            )
                            nc.vector.tensor_copy(xb[:], xf[:])
                            eng.dma_start(
                                bass.AP(xbf_dram.tensor, ck * 1024 * C,
                                        [[C, 128], [128 * C, 8], [1, C]]),
                                xb[:],
                            )
                        if not SKIP_P:
                            nc.sync.dma_start(
                                bass.AP(xbf_dram.tensor, XROWS * C, [[C, PAD], [1, C]]),
                                zpad[0:PAD, :],
                            )
                        # ---- xquad[r] = [x[r], x[r+1], x[r+64], x[r+65]] ----
                        for half in ([] if SKIP_P else range(2)):
                            ro = half * 2048
                            nc.sync.dma_start(
                                bass.AP(xquad_dram.tensor, ro * 4 * C,
                                        [[4 * C, 2048], [1, 2 * C]]),
                                bass.AP(xbf_dram.tensor, ro * C,
                                        [[C, 2048], [1, 2 * C]]),
                            )
                            nc.scalar.dma_start(
                                bass.AP(xquad_dram.tensor, ro * 4 * C + 2 * C,
                                        [[4 * C, 2048], [1, 2 * C]]),
                                bass.AP(xbf_dram.tensor, (ro + 64) * C,
                                        [[C, 2048], [1, 2 * C]]),
                            )

                        # ---- coordinates / weights / index (DVE) ----
                        nc.vector.tensor_tensor(cy[:], baseY[:], offview(0), Alu.add)
                        nc.vector.tensor_scalar(cy[:], cy[:], 0.0, float(IH - 1),
                                                Alu.max, Alu.min)
                        nc.vector.tensor_tensor(cx[:], baseX[:], offview(1), Alu.add)
                        nc.vector.tensor_scalar(cx[:], cx[:], 0.0, float(IW - 1),
                                                Alu.max, Alu.min)
                        floor_into(y0, fy, cy[:])
                        floor_into(x0, fx, cx[:])
                        nc.vector.tensor_scalar(uy[:], fy[:], -1.0, 1.0,
                                                Alu.mult, Alu.add)
                        nc.vector.tensor_scalar(vx[:], fx[:], -1.0, 1.0,
                                                Alu.mult, Alu.add)
                        nc.vector.tensor_tensor(w00[:], uy[:], vx[:], Alu.mult)
                        nc.vector.tensor_tensor(w01[:], uy[:], fx[:], Alu.mult)
                        nc.vector.tensor_tensor(w10[:], fy[:], vx[:], Alu.mult)
                        nc.vector.tensor_tensor(w11[:], fy[:], fx[:], Alu.mult)
                        nc.vector.scalar_tensor_tensor(idf[:], y0[:], float(IW),
                                                       x0[:], Alu.mult, Alu.add)
                        nc.vector.tensor_copy(idxc[:], idf[:])

                        # ---- idx -> wrapped [16, n/16] layout (DRAM trip) ----
                        # idxw[q, n*128 + j*8 + a] = idxc[16a+q, n*16+j]
                        for a in range(8):
                            nc.scalar.dma_start(
                                bass.AP(idx_dram.tensor, a,
                                        [[NW, 16], [128, N], [8, NJ]]),
                                bass.AP(idxc.tensor,
                                        idxc[:].offset + 16 * a * idxc[:].ap[0][0],
                                        [[idxc[:].ap[0][0], 16], [NJ, N], [1, NJ]]),
                            )
                        for k in range(8):
                            nc.sync.dma_start(
                                bass.AP(idxw.tensor,
                                        idxw[:].offset + 16 * k * idxw[:].ap[0][0],
                                        [[idxw[:].ap[0][0], 16], [1, NW]]),
                                bass.AP(idx_dram.tensor, 0, [[NW, 16], [1, NW]]),
                            )

                        # ---- main per-tap pipeline ----
                        call = 0
                        for jh in range(2):
                            for n in range(N):
                                dfm = dfpool.tile([128, 2, 8, 128], dt.bfloat16,
                                                  tag=f"dfm{n % 2}")
                                for sc in range(2):
                                    g = gpool.tile([128, 4, 4 * C], dt.bfloat16,
                                                   tag=f"g{call % 2}")
                                    base = n * 128 + jh * 64 + sc * 32
                                    if not SKIP_G:
                                     track(nc.gpsimd.dma_gather(
                                        out_ap=g[:],
                                        in_ap=xquadview,
                                        idxs_ap=idxw[:, base:base + 32],
                                        num_idxs=512,
                                        num_idxs_reg=512,
                                        elem_size=4 * C,
                                        elem_step=4 * C,
                                        queue_num=next_q(),
                                    ))
                                    call += 1
                                    gp = g[:].ap[0][0]
                                    goff = g[:].offset
                                    for jl4 in ([] if SKIP_B else range(4)):
                                        jl = sc * 4 + jl4
                                        j = jh * 8 + jl
                                        col = n * NJ + j
                                        dv = bass.AP(
                                            dfm.tensor,
                                            dfm[:].offset + jl * 128,
                                            [[dfm[:].ap[0][0], 128],
                                             [8 * 128, 2], [1, 128]],
                                        )

                                        def gview(corner):
                                            return bass.AP(
                                                g.tensor,
                                                goff + jl4 * 4 * C + corner * C,
                                                [[gp, 128], [128, 2], [1, 128]],
                                            )

                                        nc.scalar.activation(
                                            dv, gview(0),
                                            mybir.ActivationFunctionType.Copy,
                                            scale=w00[:, col:col + 1])
                                        nc.vector.scalar_tensor_tensor(
                                            dv, gview(1), w01[:, col:col + 1], dv,
                                            Alu.mult, Alu.add)
                                        nc.vector.scalar_tensor_tensor(
                                            dv, gview(2), w10[:, col:col + 1], dv,
                                            Alu.mult, Alu.add)
                                        nc.vector.scalar_tensor_tensor(
                                            dv, gview(3), w11[:, col:col + 1], dv,
                                            Alu.mult, Alu.add)
                                # deform^T via PE transpose (identity
                                # matmul into PSUM + copy out). Slower than
                                # the HWDGE xbar dma-transpose, but the xbar
                                # path posts completion per 16-row tile on
                                # real HW (not modeled by CoreSim), so its
                                # consumers race build-dependently; PE
                                # transposes are semaphore-sound.
                                dp = deformT[:].ap[0][0]
                                doff = deformT[:].offset
                                for jl in ([] if SKIP_T else range(8)):
                                    for ch in range(2):
                                        pst = tppool.tile([128, 128], dt.bfloat16,
                                                          tag=f"pst{jl % 2}{ch}",
                                                          name="pst")
                                        nc.tensor.transpose(
                                            pst[:],
                                            bass.AP(dfm.tensor,
                                                    dfm[:].offset
                                                    + ch * 8 * 128 + jl * 128,
                                                    [[dfm[:].ap[0][0], 128],
                                                     [1, 128]]),
                                            ident[:])
                                        nc.scalar.copy(
                                            bass.AP(deformT.tensor,
                                                    doff + ch * N * NJ * 128
                                                    + n * NJ * 128
                                                    + (jh * 8 + jl) * 128,
                                                    [[dp, 128], [1, 128]]),
                                            pst[:])

                            # ---- stage 2 for this j-half: out^T[f, px] ----
                            for ft in ([] if SKIP_M else range(2)):
                                psos = []
                                for pxt in range(2):
                                    pso = pspool.tile([128, 512], dt.float32,
                                                      tag=f"pso{ft}{pxt}",
                                                      name=f"pso{ft}{pxt}")
                                    psos.append(pso)
                                for k18 in range(2 * N):
                                    n2, ch = k18 // 2, k18 % 2
                                    lhsT = wb[:, n2, ch, ft * 128:(ft + 1) * 128]
                                    for pxt in range(2):
                                        rhs = bass.AP(
                                            deformT.tensor,
                                            doff + ch * N * NJ * 128
                                            + n2 * NJ * 128 + jh * 8 * 128
                                            + pxt * 512,
                                            [[dp, 128], [1, 512]],
                                        )
                                        nc.tensor.matmul(
                                            psos[pxt][:],
                                            lhsT=lhsT,
                                            rhs=rhs,
                                            start=(k18 == 0),
                                            stop=(k18 == 2 * N - 1),
                                        )
                                for pxt in range(2):
                                    osb = opool.tile([128, 512], dt.float32,
                                                     tag=f"osb{pxt}")
                                    nc.scalar.copy(osb[:], psos[pxt][:])
                                    nc.sync.dma_start(
                                        bass.AP(out_t,
                                                ft * 128 * PX + jh * 1024
                                                + pxt * 512,
                                                [[PX, 128], [1, 512]]),
                                        osb[:],
                                    )
    nc.compile()
    return nc


def kernel(**inputs):
    from concourse.bass_utils import run_bass_kernel_spmd

    x = np.asarray(inputs["x"], dtype=np.float32)
    offsets = np.asarray(inputs["offsets"], dtype=np.float32)
    W = np.asarray(inputs["W"], dtype=np.float32)
    b = np.asarray(inputs["b"], dtype=np.float32)

    if "nc" not in _cache:
        _cache["nc"] = _build_bass()
    nc = _cache["nc"]

    import ml_dtypes
    ident = np.eye(128).astype(ml_dtypes.bfloat16)
    in_maps = []
    for k in range(NCORES):
        bb, yh = k // 2, k % 2
        bY, bX = _host_consts(yh)
        in_maps.append({
            "x": np.ascontiguousarray(x[bb].reshape(IH * IW, C)),
            "offs": np.ascontiguousarray(
                offsets[bb, yh * HALF:(yh + 1) * HALF].reshape(PX, 2 * N)),
            "w": np.ascontiguousarray(W),
            "baseY": bY, "baseX": bX, "ident": ident,
        })

    res = run_bass_kernel_spmd(nc, in_maps, core_ids=list(range(NCORES)))
    _cache["last_result"] = res
    out = np.empty((B, IH, IW, F), dtype=np.float32)
    for k in range(NCORES):
        bb, yh = k // 2, k % 2
        outT = res.results[k]["out"]                      # [F, PX]
        out[bb, yh * HALF:(yh + 1) * HALF] = (
            outT.reshape(F, HALF, IW).transpose(1, 2, 0))
    out += b  # bias (zeros in this problem; exact elementwise add)
    return out


# revision 35
# speedup vs baseline: 7.9849x; 7.9849x over previous
"""Deformable Conv2d on 8 Trainium2 NeuronCores.

Sharding: core k -> (batch b = k//2, image row-half yh = k%2).
Each core handles 2048 output pixels (32 rows x 64 cols), all 9 taps,
full C=256 / F=256.

Per-core device pipeline (bf16 compute, f32 psum accumulation):
  1. cast x[b] -> bf16 through SBUF (DVE cast), write DRAM xbf (padded,
     zeroed tail rows), then build xquad[r] = [x[r], x[r+1], x[r+64],
     x[r+65]] (the 2x2 bilinear footprint, 2KB/row) with 4 strided
     HWDGE DMAs.
  2. coords/weights/index from offsets on DVE (exact floor with
     round-up correction; tap grid matches the reference's meshgrid
     quirk). One int16 index plane (y0*64+x0), shuffled into the
     SWDGE wrapped [16, n/16] layout via a small DRAM round trip.
  3. gpsimd.dma_gather from xquad: ONE 2KB descriptor per (pixel, tap)
     -> [128 samples, 4 corners x 256c]. 36 calls of 512 indices on a
     single SWDGE queue. Real-HW SWDGE cost is ~10ns/descriptor/queue
     (descriptor-count bound, measured), so the 2KB quad descriptors
     halve gather queue time vs the 2-descriptor scheme (~184us,
     overlapped with the rest of the pipeline). Multi-queue would cut
     this further (~31us measured) but is unsound: Tile rotates Pool
     DMAs over the 8 DMASW sem lanes in scheduled order, queue-blind,
     while the ucode hard-locks each lane to its first queue -- any
     multi-queue assignment is a per-build coin flip (W is loaded via
     HWDGE + DVE cast for the same reason).
  4. bilinear blend px-major on ACT+DVE (1 activation-copy with
     per-partition scale + 3 fused scalar_tensor_tensor MACs).
  5. deform^T via PE transpose (identity matmul into PSUM + copy out).
     The HWDGE xbar dma_start_transpose would be cheaper but posts its
     completion semaphore per xbar sub-tile on real HW (CoreSim models
     it as atomic), so stage-2 consumers race build-dependently --
     observed as intermittent j-granular corruption. PE is sound.
  6. stage 2 computes out^T: psum[f128, px512] += W[n,ch][c,f] as
     resident lhsT @ deformT[c, px] rhs (N=512 streams, 144 matmuls,
     largely hidden under the DMA/blend pipeline). Host transposes
     [F, PX] back during unshard.
Bias is added on host during unshard (zeros in this problem).
Measured on HW (slope method below): ~302us/iteration vs a 225us
TimelineSim model and a ~176us per-core HBM roofline (~63MB traffic
at 358GB/s).

`_build_bass(reps=R)` unrolls the whole per-exec pipeline R times
(shared DRAM scratch serializes reps); test.py uses an A/B pair of
such modules to measure true on-device per-iteration time as a slope,
cancelling the ~85ms axon dispatch round-trip that dominates any
single blocking execution.
"""

import numpy as np

B, IH, IW, C = 4, 64, 64, 256
KH, KW, F = 3, 3, 256
N = KH * KW
HALF = IH // 2           # 32 rows per core
PX = HALF * IW           # 2048 pixels per core
NJ = PX // 128           # 16 column-tiles of 128 pixels
NPLANE = N * NJ          # 144
NCORES = 8

_cache = {}


def _host_consts(yh):
    # base grid planes [128, N*NJ]: col (n, j), partition p, pixel = j*128+p
    # Tap grid offsets reproduce the reference's meshgrid-stack-reshape quirk.
    flat = np.array([0, 0, 0, 1, 1, 1, 2, 2, 2, 0, 1, 2, 0, 1, 2, 0, 1, 2])
    DY = flat[0::2]
    DX = flat[1::2]
    p = np.arange(128)
    j = np.arange(NJ)
    px = j[None, :] * 128 + p[:, None]          # [128, NJ] local pixel id
    Y = yh * HALF + px // IW                    # global row
    X = px % IW
    baseY = (Y[:, None, :] - 1 + DY[None, :, None]).astype(np.float32)
    baseX = (X[:, None, :] - 1 + DX[None, :, None]).astype(np.float32)
    return baseY.reshape(128, N * NJ), baseX.reshape(128, N * NJ)


def _audit_pool_lanes(nc):
    """Return (issue->scheduled-index map OK?, per-issue queue fix).

    Tile assigns Pool DMA instructions to the 8 DMASW sem lanes round-robin
    in FINAL SCHEDULED order (queue-blind), and the SWDGE ucode hard-locks
    each sem lane to the queue of its first use. So queue_num must equal
    (scheduled pool-DMA index) % 4 -- which is only knowable after
    scheduling. _build_bass records issue-ordered gather instructions in
    nc._pool_dma_insts; this reads the scheduled order back.
    """
    import concourse.mybir as mybir
    fn = nc.m.functions[0]
    insts = []
    for blk in fn.blocks:
        insts.extend(blk.instructions)
    name_to_issue = {b.ins.name: i for i, b in enumerate(nc._pool_dma_insts)}
    sched = [ins for ins in insts
             if getattr(ins, 'engine', None) == mybir.EngineType.Pool
             and type(ins).__name__ == 'InstDMAGatherAnt']
    qfix = {}
    ok = len(sched) == len(name_to_issue)
    lanes = {}
    for si, ins in enumerate(sched):
        issue = name_to_issue.get(ins.name)
        if issue is None:
            ok = False
            continue
        qfix[issue] = si % 4
        lane = si % 8
        q = getattr(ins, 'queue_num', 0)
        if lanes.setdefault(lane, q) != q:
            ok = False
    return ok, qfix


def _build_bass_fixed(reps=1):
    """Build with SWDGE queue assignments consistent with the scheduled
    sem-lane rotation (fixed-point iteration; falls back to single-queue
    if the schedule refuses to converge)."""
    qmap = None
    for _ in range(4):
        nc = _build_bass(reps=reps, qmap=qmap)
        ok, qfix = _audit_pool_lanes(nc)
        if ok:
            return nc
        qmap = qfix
    return _build_bass(reps=reps, qmap="single")


def _build_bass(reps=1, qmap=None):
    import os
    import concourse.bass as bass
    import concourse.mybir as mybir
    import concourse.tile as tile
    from concourse import bacc
    from concourse import library_config

    ABL = os.environ.get("BASS_ABLATE", "")  # timing-only stage ablations
    dt = mybir.dt
    Alu = mybir.AluOpType
    nc = bacc.Bacc(None, target_bir_lowering=False,
                   dynamic_dma_scratch_size=32768, num_swdge_queues=1)

    xin = nc.dram_tensor("x", [IH * IW, C], dt.float32, kind="ExternalInput")
    offs_in = nc.dram_tensor("offs", [PX, 2 * N], dt.float32, kind="ExternalInput")
    w_in = nc.dram_tensor("w", [N, C, F], dt.float32, kind="ExternalInput")
    baseY_in = nc.dram_tensor("baseY", [128, NPLANE], dt.float32, kind="ExternalInput")
    baseX_in = nc.dram_tensor("baseX", [128, NPLANE], dt.float32, kind="ExternalInput")
    ident_in = nc.dram_tensor("ident", [128, 128], dt.bfloat16, kind="ExternalInput")
    out_t = nc.dram_tensor("out", [F, PX], dt.float32, kind="ExternalOutput")

    XROWS = IH * IW          # 4096
    PAD = 65                 # xbf tail rows zeroed (r+65 reads for r=4095)
    NW = N * 128             # 1152 wrapped idx columns
    # (schedule-sensitive build: keep source layout stable once tuned)

    with tile.TileContext(nc) as tc:
        with tc.tile_pool(name="dram", bufs=1, space="DRAM") as dpool:
            xbf_dram = dpool.tile([XROWS + PAD, C], dt.bfloat16)
            xquad_dram = dpool.tile([XROWS, 4 * C], dt.bfloat16)
            idx_dram = dpool.tile([128 * NPLANE], dt.int16)

            with tc.tile_pool(name="main", bufs=1) as pool:
                nc.gpsimd.load_library(library_config.attnmlp)
                # Warm the Q7 library IRAM (~6us) before the real gathers.
                warm_idx = pool.tile([128, 8], dt.int16)
                warm_out = pool.tile([128, 1, 64], dt.float32)
                nc.vector.memset(warm_idx[:], 0)
                # Pool-engine DMA instructions are assigned to the 8 DMASW
                # sem lanes round-robin in program order, and each lane
                # hard-locks to the SWDGE queue of its first use. Keeping
                # queue_num == (pool-dma counter) % 4 makes lane<->queue
                # consistent (8 lanes, 4 queues). pooldma[0] is the counter.
                pooldma = [0]
                nc._pool_dma_insts = []

                def next_q():
                    # Single SWDGE queue: Tile rotates Pool DMAs over the 8
                    # DMASW sem lanes in *scheduled* order (queue-blind) and
                    # the ucode hard-locks each lane to its first queue.
                    # The schedule is not stable across builds, so any
                    # multi-queue assignment is a per-build coin flip
                    # (observed as intermittent j-granular corruption).
                    # One queue can never conflict. The gathers serialize to
                    # ~184us on the queue but overlap the rest of the
                    # pipeline.
                    pooldma[0] += 1
                    return 0

                def track(inst):
                    nc._pool_dma_insts.append(inst)
                    return inst

                track(nc.gpsimd.dma_gather(
                    out_ap=warm_out[:],
                    in_ap=bass.AP(xin, 0, [[64, 128], [1, 64]]),
                    idxs_ap=warm_idx[:],
                    num_idxs=128,
                    num_idxs_reg=128,
                    elem_size=64,
                    elem_step=64,
                    queue_num=next_q(),
                ))
                zpad = pool.tile([128, C], dt.bfloat16)
                nc.vector.memset(zpad[:], 0.0)
                ident = pool.tile([128, 128], dt.bfloat16)
                nc.sync.dma_start(ident[:], ident_in[:])

                # persistent per-rep tiles (reused across reps)
                wb = pool.tile([128, N, 2, F], dt.bfloat16)
                baseY = pool.tile([128, NPLANE], dt.float32)
                baseX = pool.tile([128, NPLANE], dt.float32)
                offs = pool.tile([128, NJ, 2 * N], dt.float32)
                cy = pool.tile([128, NPLANE], dt.float32)
                cx = pool.tile([128, NPLANE], dt.float32)
                fy = pool.tile([128, NPLANE], dt.float32)
                fx = pool.tile([128, NPLANE], dt.float32)
                y0 = pool.tile([128, NPLANE], dt.float32)
                x0 = pool.tile([128, NPLANE], dt.float32)
                uy = pool.tile([128, NPLANE], dt.float32)
                vx = pool.tile([128, NPLANE], dt.float32)
                w00 = pool.tile([128, NPLANE], dt.float32)
                w01 = pool.tile([128, NPLANE], dt.float32)
                w10 = pool.tile([128, NPLANE], dt.float32)
                w11 = pool.tile([128, NPLANE], dt.float32)
                idxc = pool.tile([128, NPLANE], dt.int16)
                idf = pool.tile([128, NPLANE], dt.float32)
                itmp = pool.tile([128, NPLANE], dt.int32)
                neg = pool.tile([128, NPLANE], dt.float32)
                idxw = pool.tile([128, NW], dt.int16)

                def offview(d):
                    return bass.AP(offs.tensor, offs[:].offset + d,
                                   [[offs[:].ap[0][0], 128], [2, N], [2 * N, NJ]])

                def floor_into(dst_i, dst_f, src):
                    nc.vector.tensor_copy(itmp[:], src)
                    nc.vector.tensor_copy(dst_i[:], itmp[:])
                    nc.vector.tensor_tensor(dst_f[:], src, dst_i[:], Alu.subtract)
                    nc.vector.tensor_scalar(neg[:], dst_f[:], 0.0, None, Alu.is_lt)
                    nc.vector.tensor_tensor(dst_i[:], dst_i[:], neg[:], Alu.subtract)
                    nc.vector.tensor_tensor(dst_f[:], dst_f[:], neg[:], Alu.add)

                with (
                    tc.tile_pool(name="xstage", bufs=1) as xpool,
                    tc.tile_pool(name="gpool", bufs=2) as gpool,
                    tc.tile_pool(name="dfpool", bufs=1) as dfpool,
                    tc.tile_pool(name="pspool", bufs=1, space="PSUM") as pspool,
                    tc.tile_pool(name="tppool", bufs=1, space="PSUM") as tppool,
                    tc.tile_pool(name="ost", bufs=2) as opool,
                ):
                    deformT = pool.tile([128, 2, N, NJ, 128], dt.bfloat16)
                    xquadview = bass.AP(xquad_dram.tensor, 0,
                                        [[4 * C, XROWS], [1, 4 * C]])

                    for rep in range(reps):
                        SKIP_G = "gather" in ABL
                        SKIP_B = "blend" in ABL
                        SKIP_T = "tpose" in ABL
                        SKIP_M = "mm" in ABL
                        SKIP_P = "prefix" in ABL
                        # ---- constant/weight loads (counted per rep) ----
                        # W: HWDGE f32 load + DVE cast (keeps Pool DMA lanes
                        # exclusively for gathers -- see next_q()).
                        for ck in range(3):
                            wf = xpool.tile([128, 6, F], dt.float32,
                                            tag=f"wf{ck % 2}")
                            nc.scalar.dma_start(
                                wf[:],
                                bass.AP(w_in, ck * 6 * 128 * F,
                                        [[F, 128], [128 * F, 6], [1, F]]),
                            )
                            nc.vector.tensor_copy(
                                bass.AP(wb.tensor,
                                        wb[:].offset + ck * 6 * F,
                                        [[wb[:].ap[0][0], 128], [F, 6], [1, F]]),
                                wf[:])
                        nc.scalar.dma_start(baseY[:], baseY_in[:])
                        nc.scalar.dma_start(baseX[:], baseX_in[:])
                        nc.scalar.dma_start(
                            offs[:],
                            bass.AP(offs_in, 0,
                                    [[2 * N, 128], [128 * 2 * N, NJ], [1, 2 * N]]),
                        )

                        # ---- x -> bf16 (DVE cast through SBUF) ----
                        for ck in ([] if SKIP_P else range(4)):
                            xf = xpool.tile([128, 8, C], dt.float32, tag=f"xf{ck % 2}")
                            xb = xpool.tile([128, 8, C], dt.bfloat16, tag=f"xb{ck % 2}")
                            eng = nc.sync if ck % 2 == 0 else nc.scalar
                            eng.dma_start(
                                xf[:],
                                bass.AP(xin, ck * 1024 * C,
                                        [[C, 128], [128 * C, 8], [1, C]]),
                            )
                            nc.vector.tensor_copy(xb[:], xf[:])
                            eng.dma_start(
                                bass.AP(xbf_dram.tensor, ck * 1024 * C,
                                        [[C, 128], [128 * C, 8], [1, C]]),
                                xb[:],
                            )
                        if not SKIP_P:
                            nc.sync.dma_start(
                                bass.AP(xbf_dram.tensor, XROWS * C, [[C, PAD], [1, C]]),
                                zpad[0:PAD, :],
                            )
                        # ---- xquad[r] = [x[r], x[r+1], x[r+64], x[r+65]] ----
                        for half in ([] if SKIP_P else range(2)):
                            ro = half * 2048
                            nc.sync.dma_start(
                                bass.AP(xquad_dram.tensor, ro * 4 * C,
                                        [[4 * C, 2048], [1, 2 * C]]),
                                bass.AP(xbf_dram.tensor, ro * C,
                                        [[C, 2048], [1, 2 * C]]),
                            )
                            nc.scalar.dma_start(
                                bass.AP(xquad_dram.tensor, ro * 4 * C + 2 * C,
                                        [[4 * C, 2048], [1, 2 * C]]),
                                bass.AP(xbf_dram.tensor, (ro + 64) * C,
                                        [[C, 2048], [1, 2 * C]]),
                            )

                        # ---- coordinates / weights / index (DVE) ----
                        nc.vector.tensor_tensor(cy[:], baseY[:], offview(0), Alu.add)
                        nc.vector.tensor_scalar(cy[:], cy[:], 0.0, float(IH - 1),
                                                Alu.max, Alu.min)
                        nc.vector.tensor_tensor(cx[:], baseX[:], offview(1), Alu.add)
                        nc.vector.tensor_scalar(cx[:], cx[:], 0.0, float(IW - 1),
                                                Alu.max, Alu.min)
                        floor_into(y0, fy, cy[:])
                        floor_into(x0, fx, cx[:])
                        nc.vector.tensor_scalar(uy[:], fy[:], -1.0, 1.0,
                                                Alu.mult, Alu.add)
                        nc.vector.tensor_scalar(vx[:], fx[:], -1.0, 1.0,
                                                Alu.mult, Alu.add)
                        nc.vector.tensor_tensor(w00[:], uy[:], vx[:], Alu.mult)
                        nc.vector.tensor_tensor(w01[:], uy[:], fx[:], Alu.mult)
                        nc.vector.tensor_tensor(w10[:], fy[:], vx[:], Alu.mult)
                        nc.vector.tensor_tensor(w11[:], fy[:], fx[:], Alu.mult)
                        nc.vector.scalar_tensor_tensor(idf[:], y0[:], float(IW),
                                                       x0[:], Alu.mult, Alu.add)
                        nc.vector.tensor_copy(idxc[:], idf[:])

                        # ---- idx -> wrapped [16, n/16] layout (DRAM trip) ----
                        # idxw[q, n*128 + j*8 + a] = idxc[16a+q, n*16+j]
                        for a in range(8):
                            nc.scalar.dma_start(
                                bass.AP(idx_dram.tensor, a,
                                        [[NW, 16], [128, N], [8, NJ]]),
                                bass.AP(idxc.tensor,
                                        idxc[:].offset + 16 * a * idxc[:].ap[0][0],
                                        [[idxc[:].ap[0][0], 16], [NJ, N], [1, NJ]]),
                            )
                        for k in range(8):
                            nc.sync.dma_start(
                                bass.AP(idxw.tensor,
                                        idxw[:].offset + 16 * k * idxw[:].ap[0][0],
                                        [[idxw[:].ap[0][0], 16], [1, NW]]),
                                bass.AP(idx_dram.tensor, 0, [[NW, 16], [1, NW]]),
                            )

                        # ---- main per-tap pipeline ----
                        call = 0
                        for jh in range(2):
                            for n in range(N):
                                dfm = dfpool.tile([128, 2, 8, 128], dt.bfloat16,
                                                  tag=f"dfm{n % 2}")
                                for sc in range(2):
                                    g = gpool.tile([128, 4, 4 * C], dt.bfloat16,
                                                   tag=f"g{call % 2}")
                                    base = n * 128 + jh * 64 + sc * 32
                                    if not SKIP_G:
                                     track(nc.gpsimd.dma_gather(
                                        out_ap=g[:],
                                        in_ap=xquadview,
                                        idxs_ap=idxw[:, base:base + 32],
                                        num_idxs=512,
                                        num_idxs_reg=512,
                                        elem_size=4 * C,
                                        elem_step=4 * C,
                                        queue_num=next_q(),
                                    ))
                                    call += 1
                                    gp = g[:].ap[0][0]
                                    goff = g[:].offset
                                    for jl4 in ([] if SKIP_B else range(4)):
                                        jl = sc * 4 + jl4
                                        j = jh * 8 + jl
                                        col = n * NJ + j
                                        dv = bass.AP(
                                            dfm.tensor,
                                            dfm[:].offset + jl * 128,
                                            [[dfm[:].ap[0][0], 128],
                                             [8 * 128, 2], [1, 128]],
                                        )

                                        def gview(corner):
                                            return bass.AP(
                                                g.tensor,
                                                goff + jl4 * 4 * C + corner * C,
                                                [[gp, 128], [128, 2], [1, 128]],
                                            )

                                        nc.scalar.activation(
                                            dv, gview(0),
                                            mybir.ActivationFunctionType.Copy,
                                            scale=w00[:, col:col + 1])
                                        nc.vector.scalar_tensor_tensor(
                                            dv, gview(1), w01[:, col:col + 1], dv,
                                            Alu.mult, Alu.add)
                                        nc.vector.scalar_tensor_tensor(
                                            dv, gview(2), w10[:, col:col + 1], dv,
                                            Alu.mult, Alu.add)
                                        nc.vector.scalar_tensor_tensor(
                                            dv, gview(3), w11[:, col:col + 1], dv,
                                            Alu.mult, Alu.add)
                                # deform^T via PE transpose (identity
                                # matmul into PSUM + copy out). Slower than
                                # the HWDGE xbar dma-transpose, but the xbar
                                # path posts completion per 16-row tile on
                                # real HW (not modeled by CoreSim), so its
                                # consumers race build-dependently; PE
                                # transposes are semaphore-sound.
                                dp = deformT[:].ap[0][0]
                                doff = deformT[:].offset
                                for jl in ([] if SKIP_T else range(8)):
                                    for ch in range(2):
                                        pst = tppool.tile([128, 128], dt.bfloat16,
                                                          tag=f"pst{jl % 2}{ch}",
                                                          name="pst")
                                        nc.tensor.transpose(
                                            pst[:],
                                            bass.AP(dfm.tensor,
                                                    dfm[:].offset
                                                    + ch * 8 * 128 + jl * 128,
                                                    [[dfm[:].ap[0][0], 128],
                                                     [1, 128]]),
                                            ident[:])
                                        nc.scalar.copy(
                                            bass.AP(deformT.tensor,
                                                    doff + ch * N * NJ * 128
                                                    + n * NJ * 128
                                                    + (jh * 8 + jl) * 128,
                                                    [[dp, 128], [1, 128]]),
                                            pst[:])

                            # ---- stage 2 for this j-half: out^T[f, px] ----
                            for ft in ([] if SKIP_M else range(2)):
                                psos = []
                                for pxt in range(2):
                                    pso = pspool.tile([128, 512], dt.float32,
                                                      tag=f"pso{ft}{pxt}",
                                                      name=f"pso{ft}{pxt}")
                                    psos.append(pso)
                                for k18 in range(2 * N):
                                    n2, ch = k18 // 2, k18 % 2
                                    lhsT = wb[:, n2, ch, ft * 128:(ft + 1) * 128]
                                    for pxt in range(2):
                                        rhs = bass.AP(
                                            deformT.tensor,
                                            doff + ch * N * NJ * 128
                                            + n2 * NJ * 128 + jh * 8 * 128
                                            + pxt * 512,
                                            [[dp, 128], [1, 512]],
                                        )
                                        nc.tensor.matmul(
                                            psos[pxt][:],
                                            lhsT=lhsT,
                                            rhs=rhs,
                                            start=(k18 == 0),
                                            stop=(k18 == 2 * N - 1),
                                        )
                                for pxt in range(2):
                                    osb = opool.tile([128, 512], dt.float32,
                                                     tag=f"osb{pxt}")
                                    nc.scalar.copy(osb[:], psos[pxt][:])
                                    nc.sync.dma_start(
                                        bass.AP(out_t,
                                                ft * 128 * PX + jh * 1024
                                                + pxt * 512,
                                                [[PX, 128], [1, 512]]),
                                        osb[:],
                                    )
    nc.compile()
    return nc


def kernel(**inputs):
    from concourse.bass_utils import run_bass_kernel_spmd

    x = np.asarray(inputs["x"], dtype=np.float32)
    offsets = np.asarray(inputs["offsets"], dtype=np.float32)
    W = np.asarray(inputs["W"], dtype=np.float32)
    b = np.asarray(inputs["b"], dtype=np.float32)

    if "nc" not in _cache:
        _cache["nc"] = _build_bass()
    nc = _cache["nc"]

    import ml_dtypes
    ident = np.eye(128).astype(ml_dtypes.bfloat16)
    in_maps = []
    for k in range(NCORES):
        bb, yh = k // 2, k % 2
        bY, bX = _host_consts(yh)
        in_maps.append({
            "x": np.ascontiguousarray(x[bb].reshape(IH * IW, C)),
            "offs": np.ascontiguousarray(
                offsets[bb, yh * HALF:(yh + 1) * HALF].reshape(PX, 2 * N)),
            "w": np.ascontiguousarray(W),
            "baseY": bY, "baseX": bX, "ident": ident,
        })

    res = run_bass_kernel_spmd(nc, in_maps, core_ids=list(range(NCORES)))
    _cache["last_result"] = res
    out = np.empty((B, IH, IW, F), dtype=np.float32)
    for k in range(NCORES):
        bb, yh = k // 2, k % 2
        outT = res.results[k]["out"]                      # [F, PX]
        out[bb, yh * HALF:(yh + 1) * HALF] = (
            outT.reshape(F, HALF, IW).transpose(1, 2, 0))
    out += b  # bias (zeros in this problem; exact elementwise add)
    return out
